# revision 10
# baseline (speedup 1.0000x reference)
"""Multi-head attention (B=2, S=2048, D=1024, H=16) on 8 Trainium2 NeuronCores.

Sharding: core c handles batch b = c//4 and head-group g = c%4 (4 heads,
a 256-wide column slice of wq/wk/wv and row slice of wo).  Each core
computes a full [S, D] partial of the output projection (bf16); the host
sums the 4 partials per batch in fp32 and adds the output bias.

v2 schedule (ACT-exp is the floor at ~147us; keep it saturated):
  - chunked input DMA ordered so K-proj starts at ~5us and the attention
    loop (and with it the scalar-engine exp stream) starts at ~16us.
  - remaining K-proj chunks, all of V-proj, Q-proj for sq1..3, the
    output projection and the normalization all run as deadline-scheduled
    PE filler units inside the exp-bound attention steps.
  - the 1-lane DVE reciprocals are deferred one step and ordered so the
    ctx-PSUM evacuation copies are always at the head of the DVE queue:
    no PE stall at step boundaries, HAM clock gate stays at 8/8.
"""

import os
import sys

import ml_dtypes
import numpy as np

if "/opt/trn_rl_repo" not in sys.path:
    sys.path.insert(0, "/opt/trn_rl_repo")

B, S, D, H = 2, 2048, 1024, 16
DH = D // H  # 64
NCORES = 8
GC = 256  # column slice per core (4 heads)
NP = 2  # head pairs per core
KC = D // 128  # 8 contraction chunks
SQC = S // 512  # 4 query chunks
SKC = S // 128  # 16 key chunks
NSTEP = NP * SQC  # 8 attention steps

_CACHE = {}


def _build_program():
    import concourse.bass as bass
    import concourse.tile as tile
    from concourse import bacc, mybir

    F32 = mybir.dt.float32
    F32R = mybir.dt.float32r
    BF16 = mybir.dt.bfloat16
    EXP = mybir.ActivationFunctionType.Exp
    PSUM = bass.MemorySpace.PSUM

    nc = bacc.Bacc()

    qT = nc.dram_tensor("qT", (D, S), BF16, kind="ExternalInput").ap()
    kT = nc.dram_tensor("kT", (D, S), BF16, kind="ExternalInput").ap()
    vT = nc.dram_tensor("vT", (D, S), BF16, kind="ExternalInput").ap()
    wqs = nc.dram_tensor("wqs", (D, GC), BF16, kind="ExternalInput").ap()
    wks = nc.dram_tensor("wks", (D, GC), BF16, kind="ExternalInput").ap()
    wvs = nc.dram_tensor("wvs", (D, GC), BF16, kind="ExternalInput").ap()
    wos = nc.dram_tensor("wos", (GC, D), BF16, kind="ExternalInput").ap()
    bqs = nc.dram_tensor("bqs", (NP, 128, 1), F32, kind="ExternalInput").ap()
    bks = nc.dram_tensor("bks", (NP, 128, 1), F32, kind="ExternalInput").ap()
    bvs = nc.dram_tensor("bvs", (1, GC), F32R, kind="ExternalInput").ap()
    outp = nc.dram_tensor("outp", (S, D), BF16, kind="ExternalOutput").ap()

    with tile.TileContext(nc) as tc:
        with (
            tc.tile_pool(name="const", bufs=1) as const,
            tc.tile_pool(name="raw", bufs=1) as rawp,
            tc.tile_pool(name="probs", bufs=6) as probs,
            tc.tile_pool(name="small", bufs=2) as small,
            tc.tile_pool(name="outsb", bufs=3) as outsb,
            tc.tile_pool(name="pproj", bufs=2, space=PSUM) as pproj,
            tc.tile_pool(name="psc", bufs=2, space=PSUM) as psc,
            tc.tile_pool(name="pctx", bufs=1, space=PSUM) as pctx,
        ):
            # ---- persistent tiles ----
            wq_t = const.tile([128, KC, GC], BF16, name="wqt", tag="wqt")
            wk_t = const.tile([128, KC, GC], BF16, name="wkt", tag="wkt")
            wv_t = const.tile([128, KC, GC], BF16, name="wvt", tag="wvt")
            wo_t = [const.tile([128, D], BF16, name=f"wo{m}", tag=f"wo{m}") for m in range(NP)]
            bq_t = [const.tile([128, 1], F32, name=f"bq{m}", tag=f"bq{m}", padded_shape=[128, 128]) for m in range(NP)]
            bk_t = [const.tile([128, 1], F32, name=f"bk{m}", tag=f"bk{m}", padded_shape=[128, 128]) for m in range(NP)]
            bv_row = const.tile([1, GC], F32R, name="bvrow", tag="bvrow")
            ones_t = const.tile([128, 128], F32R, name="ones", tag="ones")

            QT = [const.tile([128, S], BF16, name=f"QT{m}", tag=f"QT{m}") for m in range(NP)]
            KT = [const.tile([128, S], BF16, name=f"KT{m}", tag=f"KT{m}") for m in range(NP)]
            VH = [const.tile([128, 4, 66], BF16, name=f"VH{i}", tag=f"VH{i}", padded_shape=[128, 4, 128]) for i in range(SKC)]
            ctxT = [const.tile([128, S], BF16, name=f"ctxT{m}", tag=f"ctxT{m}") for m in range(NP)]

            # per-chunk raw input tiles (separate tags: precise DMA deps)
            k_raw = [rawp.tile([128, KC, 512], BF16, name=f"kraw{j}", tag=f"kraw{j}") for j in range(4)]
            q_raw = [rawp.tile([128, KC, 512], BF16, name=f"qraw{j}", tag=f"qraw{j}") for j in range(4)]
            v_raw = [rawp.tile([128, KC, 512], BF16, name=f"vraw{j}", tag=f"vraw{j}") for j in range(4)]

            # ---- PE warmup chain: keep HAM busy until K-proj starts ----
            wu = const.tile([128, 512], BF16, name="wu", tag="wu")
            nc.vector.memset(wu[:], 0.0)
            wup = psc.tile([128, 1024], F32, name="sc", tag="sc")
            for w in range(6):
                nc.tensor.matmul(
                    wup[:, 0:512], wu[:, 0:128], wu[:],
                    start=(w == 0), stop=(w == 5),
                )
            # pre-load the exp table set while DMAs stream
            scrap = const.tile([1, 16], BF16, name="scrap", tag="scrap", padded_shape=[1, 256])
            nc.scalar.activation(scrap[:], wu[0:1, 0:16], EXP, scale=0.125)

            # ---- input DMAs, in queue order tuned to the schedule ----
            kR = kT.rearrange("(c p) s -> p c s", p=128)
            qR = qT.rearrange("(c p) s -> p c s", p=128)
            vR = vT.rearrange("(c p) s -> p c s", p=128)

            def chunk_dma(dst, srcR, j):
                nc.sync.dma_start(dst[:], srcR[:, :, j * 512:(j + 1) * 512])

            # small tensors ride the (otherwise idle) gpsimd SWDGE ring
            for m in range(NP):
                nc.gpsimd.dma_start(bq_t[m][:], bqs[m])
                nc.gpsimd.dma_start(bk_t[m][:], bks[m])
            nc.gpsimd.dma_start(bv_row[:], bvs[:])
            # main ring: ordered by first-use time in the schedule
            nc.sync.dma_start(wk_t[:], wks.rearrange("(c p) g -> p c g", p=128))
            chunk_dma(k_raw[0], kR, 0)
            nc.sync.dma_start(wq_t[:], wqs.rearrange("(c p) g -> p c g", p=128))
            chunk_dma(q_raw[0], qR, 0)
            nc.sync.dma_start(wv_t[:], wvs.rearrange("(c p) g -> p c g", p=128))
            chunk_dma(v_raw[0], vR, 0)
            chunk_dma(k_raw[1], kR, 1)
            chunk_dma(v_raw[1], vR, 1)
            chunk_dma(k_raw[2], kR, 2)
            chunk_dma(v_raw[2], vR, 2)
            chunk_dma(k_raw[3], kR, 3)
            chunk_dma(v_raw[3], vR, 3)
            chunk_dma(q_raw[1], qR, 1)
            chunk_dma(q_raw[2], qR, 2)
            chunk_dma(q_raw[3], qR, 3)
            for m in range(NP):
                nc.sync.dma_start(wo_t[m][:], wos[m * 128:(m + 1) * 128, :])

            ones_f = const.tile([128, 128], F32, name="onesf", tag="onesf")
            nc.vector.memset(ones_f[:], 1.0)
            nc.vector.tensor_copy(ones_t[:], ones_f[:])
            for i in range(SKC):
                nc.vector.memset(VH[i][:, :, 64:66], 1.0)

            # bv broadcast to all partitions via ones-matmul
            bvb = const.tile([128, GC], F32, name="bvb", tag="bvb")
            bvp = pproj.tile([128, 512], F32, name="pj", tag="pj")
            nc.tensor.matmul(
                bvp[:, :GC], ones_t[0:1, 0:128], bv_row[:],
                start=True, stop=True,
            )
            nc.vector.tensor_copy(bvb[:], bvp[:, :GC])

            # ---- proj unit emitters ----
            def qk_half(raw_j, w_t, b_t, dst, m, sq, h):
                # one complete accumulation: [128, 256] output columns
                def emit():
                    ps = pproj.tile([128, 512], F32, name="pj", tag="pj")
                    c0 = sq * 512 + h * 256
                    for k in range(KC):
                        nc.tensor.matmul(
                            ps[:, 0:256],
                            w_t[:, k, m * 128:(m + 1) * 128],
                            raw_j[:, k, h * 256:(h + 1) * 256],
                            start=(k == 0),
                            stop=(k == KC - 1),
                        )
                    nc.vector.tensor_scalar_add(dst[:, c0:c0 + 256], ps[:, 0:256], b_t[:])
                return emit

            def kproj_group(nq, m):
                # K-proj in full 512-col groups (pre-attention, PE-dense)
                def emit():
                    ps = pproj.tile([128, 512], F32, name="pj", tag="pj")
                    for k in range(KC):
                        nc.tensor.matmul(
                            ps[:],
                            wk_t[:, k, m * 128:(m + 1) * 128],
                            k_raw[nq][:, k, :],
                            start=(k == 0),
                            stop=(k == KC - 1),
                        )
                    nc.vector.tensor_scalar_add(
                        KT[m][:, nq * 512:(nq + 1) * 512], ps[:], bk_t[m][:]
                    )
                return emit

            def vh_unit(i):
                def emit():
                    ps = pproj.tile([128, 512], F32, name="pj", tag="pj")
                    for k in range(KC):
                        nc.tensor.matmul(
                            ps[:, :GC],
                            v_raw[i // 4][:, k, (i % 4) * 128:(i % 4 + 1) * 128],
                            wv_t[:, k, :],
                            start=(k == 0),
                            stop=(k == KC - 1),
                        )
                    nc.vector.tensor_add(
                        VH[i][:, :, 0:64],
                        ps[:, :GC].rearrange("p (h d) -> p h d", h=4),
                        bvb[:].rearrange("p (h d) -> p h d", h=4),
                    )
                return emit

            def outproj_unit(sq128, ncol):
                def emit():
                    po = pproj.tile([128, 512], F32, name="pj", tag="pj")
                    for m in range(NP):
                        nc.tensor.matmul(
                            po[:],
                            ctxT[m][:, sq128 * 128:(sq128 + 1) * 128],
                            wo_t[m][:, ncol * 512:(ncol + 1) * 512],
                            start=(m == 0),
                            stop=(m == NP - 1),
                        )
                    ob = outsb.tile([128, 512], BF16, name="ob", tag="ob")
                    nc.vector.tensor_copy(ob[:], po[:])
                    nc.sync.dma_start(
                        outp[sq128 * 128:(sq128 + 1) * 128,
                             ncol * 512:(ncol + 1) * 512],
                        ob[:],
                    )
                return emit

            # ---- startup compute: only what step 0 needs (m0) ----
            kproj_group(0, 0)()
            for h in range(2):
                qk_half(q_raw[0], wq_t, bq_t[0], QT[0], 0, 0, h)()

            # ---- filler queues ----
            # fill_pre: units that must precede this iteration's SCORES
            #   (K-proj / Q-proj); popped before the scores matmuls.
            # fill_post: units only needed by this iteration's CTX matmul
            #   or later (V-proj, outproj); popped between the exp and the
            #   ctx matmuls so a DMA-starved unit never delays exp.
            # Entries: (emit_fn, deadline, release); deadline = (step, it)
            # by which the unit MUST be emitted; release = earliest (step,
            # it) at which it MAY be popped by the drain rule.
            fill_pre = []
            fill_post = []
            fill_pre.append((kproj_group(1, 0), (0, 2), None))
            fill_pre.append((kproj_group(1, 1), (0, 3), None))
            fill_pre.append((kproj_group(0, 1), (0, 13), None))
            fill_pre.append(
                (qk_half(q_raw[0], wq_t, bq_t[1], QT[1], 1, 0, 0), (0, 14), None)
            )
            fill_pre.append(
                (qk_half(q_raw[0], wq_t, bq_t[1], QT[1], 1, 0, 1), (0, 15), None)
            )
            fill_pre.append((kproj_group(2, 0), (0, 6), None))
            fill_pre.append((kproj_group(2, 1), (0, 7), None))
            fill_pre.append((kproj_group(3, 0), (0, 10), None))
            fill_pre.append((kproj_group(3, 1), (0, 11), None))
            for sq in range(1, SQC):
                for m in range(NP):
                    for h in range(2):
                        fill_pre.append(
                            (qk_half(q_raw[sq], wq_t, bq_t[m], QT[m], m, sq, h),
                             (sq * NP + m - 1, 13), None)
                        )
            for j in range(SKC):
                fill_post.append((vh_unit(j), (0, j), None))

            def _pop(queue, step_idx, it, drain):
                popped = 0
                idx = 0
                while idx < len(queue):
                    emit_fn, dl, _rel = queue[idx]
                    if dl is not None and (
                        dl[0] < step_idx or (dl[0] == step_idx and dl[1] <= it)
                    ):
                        queue.pop(idx)
                        emit_fn()
                        popped += 1
                    else:
                        idx += 1
                if drain and popped == 0:
                    for idx in range(len(queue)):
                        emit_fn, _dl, rel = queue[idx]
                        if rel is None or (
                            rel[0] < step_idx
                            or (rel[0] == step_idx and rel[1] <= it)
                        ):
                            queue.pop(idx)
                            emit_fn()
                            break

            # ---- attention steps ----
            def attn_step(sq, m, step_idx, prev):
                # prev = (psq, pm, ctsb, r_t) of the previous step; its
                # reciprocal work is spread through THIS step's iterations
                # in 64-element chunks so it never head-of-line-blocks the
                # DVE FIFO; the normalize multiplies run at i == 9.
                ctA = pctx.tile([128, 512], F32, name="ctA", tag="ctA")
                ctB = pctx.tile([128, 512], F32, name="ctB", tag="ctB")
                for i in range(SKC):
                    _pop(fill_pre, step_idx, i, drain=False)
                    if prev is not None and i <= 7:
                        # 16 reciprocal chunks over iterations 0..7; they
                        # finish (incl. DVE backlog) well before the norm
                        # multiplies at i == 13 consume them
                        _, _, pctsb, prt = prev
                        with nc.allow_low_precision(
                            reason="f32r == fp32 bits; PE-read rounding only"
                        ):
                            for c in range(i * 2, i * 2 + 2):
                                nc.vector.reciprocal(
                                    prt[64:65, c * 64:(c + 1) * 64],
                                    pctsb[64:65, c * 64:(c + 1) * 64],
                                )
                    if i == 13 and prev is not None:
                        norm_finish(*prev)
                    sc = psc.tile([128, 1024], F32, name="sc", tag="sc")
                    nc.tensor.matmul(
                        sc[:, 0:512],
                        KT[m][0:64, i * 128:(i + 1) * 128],
                        QT[m][0:64, sq * 512:(sq + 1) * 512],
                        start=True, stop=True,
                    )
                    nc.tensor.matmul(
                        sc[:, 512:1024],
                        KT[m][64:128, i * 128:(i + 1) * 128],
                        QT[m][64:128, sq * 512:(sq + 1) * 512],
                        start=True, stop=True,
                        tile_position=(64, 0),
                    )
                    pb = probs.tile([128, 1024], BF16, name="pb", tag="pb")
                    nc.scalar.activation(pb[:], sc[:], EXP, scale=0.125)
                    _pop(fill_post, step_idx, i, drain=(i >= 2))
                    nc.tensor.matmul(
                        ctA[0:65, :], VH[i][:, 2 * m, 0:65], pb[:, 0:512],
                        start=(i == 0), stop=(i == SKC - 1),
                    )
                    nc.tensor.matmul(
                        ctB[0:65, :], VH[i][:, 2 * m + 1, 0:65], pb[:, 512:1024],
                        start=(i == 0), stop=(i == SKC - 1),
                    )
                # evacuate ctx PSUM immediately (head of DVE queue at the
                # step boundary -> next step's ctx matmuls never stall)
                ctsb = small.tile([65, 1024], F32, name="ctsb", tag="ctsb")
                nc.vector.tensor_copy(ctsb[0:65, 0:512], ctA[0:65, :])
                nc.vector.tensor_copy(ctsb[0:65, 512:1024], ctB[0:65, :])
                r_t = small.tile([65, 1024], F32R, name="rt", tag="rt")
                return ctsb, r_t

            def norm_finish(sq, m, ctsb, r_t):
                # broadcast 1/Z to 64 partitions, then normalize ctx
                rpA = pproj.tile([128, 512], F32, name="pj", tag="pj")
                nc.tensor.matmul(
                    rpA[0:64, :], ones_t[64:65, 0:64], r_t[64:65, 0:512],
                    start=True, stop=True, tile_position=(64, 0),
                )
                nc.vector.tensor_mul(
                    ctxT[m][0:64, sq * 512:(sq + 1) * 512],
                    ctsb[0:64, 0:512],
                    rpA[0:64, :],
                )
                rpB = pproj.tile([128, 512], F32, name="pj", tag="pj")
                nc.tensor.matmul(
                    rpB[0:64, :], ones_t[64:65, 0:64], r_t[64:65, 512:1024],
                    start=True, stop=True, tile_position=(64, 0),
                )
                stg = small.tile([64, 512], BF16, name="stg", tag="stg")
                nc.vector.tensor_mul(stg[:], ctsb[0:64, 512:1024], rpB[0:64, :])
                nc.sync.dma_start(
                    ctxT[m][64:128, sq * 512:(sq + 1) * 512], stg[:]
                )

            pending = None
            for step in range(NSTEP):
                sq, m = step // NP, step % NP
                state = attn_step(sq, m, step, pending)
                if pending is not None and pending[1] == NP - 1:
                    # ctxT[psq] complete once the deferred norm (emitted at
                    # i==9 of this step) runs: release its outproj units
                    psq = pending[0]
                    for sq128 in range(psq * 4, (psq + 1) * 4):
                        for ncol in range(D // 512):
                            fill_post.append(
                                (outproj_unit(sq128, ncol), None, (step + 1, 2))
                            )
                pending = (sq, m, *state)

            # tail: final reciprocal (split per head so outproj can chase),
            # final norm, remaining outproj
            psq, pm, pctsb, prt = pending
            with nc.allow_low_precision(
                reason="f32r == fp32 bits; PE-read rounding only"
            ):
                for c in range(16):
                    nc.vector.reciprocal(
                        prt[64:65, c * 64:(c + 1) * 64],
                        pctsb[64:65, c * 64:(c + 1) * 64],
                    )
            norm_finish(psq, pm, pctsb, prt)
            for q2 in (fill_pre, fill_post):
                while q2:
                    emit_fn, _dl, _rel = q2.pop(0)
                    emit_fn()
            for sq128 in range(psq * 4, (psq + 1) * 4):
                for ncol in range(D // 512):
                    outproj_unit(sq128, ncol)()

    nc.compile()
    return nc


def get_program():
    if "nc" not in _CACHE:
        _CACHE["nc"] = _build_program()
    return _CACHE["nc"]


def make_in_maps(q, k, v, wq, bq, wk, bk, wv, bv, wo, bo):
    q, k, v = (np.asarray(x, np.float32) for x in (q, k, v))
    wq, wk, wv, wo = (np.asarray(x, np.float32) for x in (wq, wk, wv, wo))
    bq, bk, bv = (np.asarray(x, np.float32) for x in (bq, bk, bv))
    BF = ml_dtypes.bfloat16
    qT = [np.ascontiguousarray(q[b].T).astype(BF) for b in range(B)]
    kTt = [np.ascontiguousarray(k[b].T).astype(BF) for b in range(B)]
    vTt = [np.ascontiguousarray(v[b].T).astype(BF) for b in range(B)]
    in_maps = []
    for c in range(NCORES):
        b, g = c // 4, c % 4
        sl = slice(g * GC, (g + 1) * GC)
        in_maps.append(
            {
                "qT": qT[b],
                "kT": kTt[b],
                "vT": vTt[b],
                "wqs": np.ascontiguousarray(wq[:, sl]).astype(BF),
                "wks": np.ascontiguousarray(wk[:, sl]).astype(BF),
                "wvs": np.ascontiguousarray(wv[:, sl]).astype(BF),
                "wos": np.ascontiguousarray(wo[sl, :]).astype(BF),
                "bqs": np.ascontiguousarray(bq[sl]).reshape(NP, 128, 1),
                "bks": np.ascontiguousarray(bk[sl]).reshape(NP, 128, 1),
                "bvs": np.ascontiguousarray(bv[sl]).reshape(1, GC),
            }
        )
    return in_maps


def combine_outputs(results, bo):
    out = np.zeros((B, S, D), np.float32)
    for c in range(NCORES):
        out[c // 4] += np.asarray(results[c]["outp"], np.float32)
    out += np.asarray(bo, np.float32)
    return out


def kernel(q, k, v, wq, bq, wk, bk, wv, bv, wo, bo, trace=False):
    from concourse.bass_utils import run_bass_kernel_spmd

    nc = get_program()
    in_maps = make_in_maps(q, k, v, wq, bq, wk, bk, wv, bv, wo, bo)
    res = run_bass_kernel_spmd(nc, in_maps, list(range(NCORES)), trace=trace)
    out = combine_outputs(res.results, bo)
    if trace:
        _CACHE["last_result"] = res
    return out
